# revision 25
# baseline (speedup 1.0000x reference)
"""GAT (2-layer, 8-head then 1-head) on 8 Trainium2 NeuronCores.

Strategy
--------
Edges (+self-loops) are sorted by destination and dst-sharded across the 8
cores (6250 dst nodes per core). Per core, destinations are processed in
windows of 128 dsts; each window's edges are split by source-table half
(dma_gather indices are int16) and padded to blocks of 128 edges.

Per layer:
  Phase A (dense): H = X @ [W | W@a_src] -> rows [h(256) | es(H) | pad]
    (768-byte bf16 rows; the pad is forced by dma_gather's 256B element
    granularity). Phase A2: ED_local = X_slice @ (W@a_dst) per-core table
    of dst logits, indexed core-locally so indices fit int16.
  Phase B (edges), per 128-dst window (one batched pipeline):
    - dma_gather of all the window's edge rows H[src[e]] (one instruction
      per table half) and ED_local[dst[e]] -> [128, nb, *] tiles
    - p = exp(leakyrelu(es + ed, 0.2)) batched over the window's blocks
    - per 128-edge block: one-hot S01[e, d] = (iota[d] == dst_local[e])
      built in ONE tensor_scalar op (layer 2 folds p in via the fused
      second scalar op), then TensorE: psum[128 dst, C+H] += S01^T @
      [p*h | p]  (segment-sum on the PE, accumulated across blocks)
    - epilogue: out = (sum p*h) / (sum p), + bias, ELU. No segment max is
      needed: logits are O(+-10) so exp() is fp32-safe.

The softmax division uses sum(p*h)/sum(p) == sum((p/sum p) * h), which
makes the edge phase single-pass. Layer-1 h is laid out channel-major
(c*H+hd) so the per-edge p-broadcast multiply hits the DVE 2x mode; the
epilogue permutes back via access patterns.

Between the two layers the per-core z slices are gathered on the host (two
NEFF launches); z^T is layer 2's dense-phase lhsT.
"""

import math
import os

import numpy as np
import ml_dtypes

import concourse.bass as bass
import concourse.tile as tile
from concourse import bacc, mybir
from concourse.bass_utils import run_bass_kernel_spmd

P = 128
NCORES = 8
NEG_SLOPE = 0.2
HALF = 32768          # dma_gather int16 index limit -> split H table rows
F32 = mybir.dt.float32
BF16 = mybir.dt.bfloat16
I16 = mybir.dt.int16
NPBF = ml_dtypes.bfloat16

LAST_EXEC_NS = []


# --------------------------------------------------------------------------
# host-side preprocessing
# --------------------------------------------------------------------------

def _pack16(a):
    """[nb, 128] per-block values -> [128, nb*8] dma_gather index layout
    (index i of a batch at [i%16, i//16], replicated across the 8 Q7 cores).
    """
    nb = a.shape[0]
    out = a.reshape(nb, 8, 16).transpose(2, 0, 1).reshape(16, nb * 8)
    return np.ascontiguousarray(np.tile(out, (8, 1)))


def _preprocess_edges(src, dst, n_nodes, dstn, half=None):
    """Sort by dst, shard, window by 128 dsts, split by src half, pad to
    128-edge blocks (uniform across cores).

    Returns (per_core, wins, groups, tb):
      per_core: dicts with gidx[128, tb*8]i16, edidx[128, tb*8]i16,
                dstloc[128, tb]bf16
      wins: [(nb_lo, nb_hi), ...] per window (shared across cores)
      groups: [(w0, w1, jblk0, nblk), ...] meta-chunk grouping
    """
    if half is None:
        half = HALF
    order = np.argsort(dst, kind="stable")
    src = np.asarray(src)[order]
    dst = np.asarray(dst)[order]
    nw = math.ceil(dstn / P)

    # per (core, window): edge ranges and lo/hi counts
    lists = [[None] * nw for _ in range(NCORES)]
    for k in range(NCORES):
        for w in range(nw):
            lo_b = k * dstn + w * P
            hi_b = min(k * dstn + min((w + 1) * P, dstn), n_nodes)
            s = np.searchsorted(dst, lo_b, side="left")
            e = np.searchsorted(dst, hi_b, side="left")
            sw = src[s:e]
            dw = dst[s:e]
            m = sw < half
            lists[k][w] = (sw[m], dw[m], sw[~m], dw[~m])

    wins = []
    for w in range(nw):
        nb_lo = max(math.ceil(len(lists[k][w][0]) / P) for k in range(NCORES))
        nb_hi = max(math.ceil(len(lists[k][w][2]) / P) for k in range(NCORES))
        wins.append((nb_lo, nb_hi))
    tb = int(sum(a + b for a, b in wins))

    # meta groups of <= 128 blocks, aligned to window boundaries
    groups = []
    w0, j0, acc = 0, 0, 0
    for w in range(nw):
        nb = wins[w][0] + wins[w][1]
        if acc + nb > 128 and acc > 0:
            groups.append((w0, w, j0, acc))
            w0, j0, acc = w, j0 + acc, 0
        acc += nb
    groups.append((w0, nw, j0, acc))

    per_core = []
    for k in range(NCORES):
        gidx = np.zeros((tb, P), dtype=np.int16)
        edidx = np.zeros((tb, P), dtype=np.int16)
        dstloc = np.full((tb, P), 255.0, dtype=np.float32)
        j = 0
        for w in range(nw):
            nb_lo, nb_hi = wins[w]
            win_base = k * dstn + w * P
            for (ss, dd), nb in ((lists[k][w][:2], nb_lo),
                                 (lists[k][w][2:], nb_hi)):
                if nb == 0:
                    continue
                cnt = len(ss)
                bs = np.zeros(nb * P, dtype=np.int64)
                bd = np.zeros(nb * P, dtype=np.int64)
                bl = np.full(nb * P, 255.0, dtype=np.float32)
                bs[:cnt] = ss
                bd[:cnt] = dd
                bl[:cnt] = dd - win_base
                bs[:cnt] -= (bs[:cnt] >= half) * half
                gidx[j:j + nb] = bs.reshape(nb, P).astype(np.int16)
                edidx[j:j + nb] = (bd.reshape(nb, P)
                                   - k * dstn).astype(np.int16)
                edidx[j:j + nb][bl.reshape(nb, P) == 255.0] = 0
                dstloc[j:j + nb] = bl.reshape(nb, P)
                j += nb
        assert j == tb
        per_core.append(dict(
            gidx=_pack16(gidx),
            edidx=_pack16(edidx),
            dstloc=np.ascontiguousarray(dstloc.T).astype(NPBF),
        ))
    return per_core, wins, groups, tb


def _fold_weights(W, a_src, a_dst):
    """[W(channel-major per head if heads>1) | W@a_src | W@a_dst] in f64."""
    fin, fout = W.shape
    heads, ch = a_src.shape
    assert heads * ch == fout
    W64 = W.astype(np.float64)
    Wh = W64.reshape(fin, heads, ch)
    ws = np.einsum("fhc,hc->fh", Wh, a_src.astype(np.float64))
    wd = np.einsum("fhc,hc->fh", Wh, a_dst.astype(np.float64))
    if heads > 1:
        # channel-major: col c*heads+hd <- (hd, c)
        Wcm = Wh.transpose(0, 2, 1).reshape(fin, fout)
    else:
        Wcm = W64
    return np.concatenate([Wcm, ws, wd], axis=1)


# --------------------------------------------------------------------------
# NEFF builder (one layer)
# --------------------------------------------------------------------------

def _build_layer(*, n_pad, in_k, c_out, heads, tb, wins, groups, dstn,
                 has_bias, out_dt, work_mult=1):
    dcol = c_out + heads          # matmul rhs/psum width
    gcol = 384 if in_k != 0 else 0   # gathered row width (256B granules)
    assert c_out + heads <= gcol
    wcols = c_out + 2 * heads
    mt = n_pad // P
    kt = in_k // P
    dstn_pad = math.ceil(dstn / P) * P
    wt = dstn_pad // P
    nw = len(wins)
    chm = heads > 1               # channel-major h layout

    nc = bacc.Bacc("TRN2", target_bir_lowering=False, debug=False,
                   num_devices=NCORES)

    xT = nc.dram_tensor("xT", [in_k, n_pad], BF16, kind="ExternalInput").ap()
    xsT = nc.dram_tensor("xsliceT", [in_k, dstn_pad], BF16,
                         kind="ExternalInput").ap()
    wext = nc.dram_tensor("wext", [in_k, wcols], BF16,
                          kind="ExternalInput").ap()
    iota_d = nc.dram_tensor("iota", [P, P], BF16, kind="ExternalInput").ap()
    iden_d = nc.dram_tensor("iden", [P, P], BF16, kind="ExternalInput").ap()
    gidx_d = nc.dram_tensor("gidx", [P, tb * 8], I16,
                            kind="ExternalInput").ap()
    loc_d = nc.dram_tensor("dstloc", [P, tb], BF16,
                           kind="ExternalInput").ap()
    bias_b = None
    if has_bias:
        bias_b = nc.dram_tensor("bias_b", [P, c_out], F32,
                                kind="ExternalInput").ap()

    hi_rows = n_pad - HALF
    hext_lo = nc.dram_tensor("hext_lo", [HALF, gcol], BF16,
                             kind="Internal").ap()
    hext_hi = nc.dram_tensor("hext_hi", [hi_rows, gcol], BF16,
                             kind="Internal").ap()
    edl = nc.dram_tensor("edl", [dstn_pad, 128], BF16, kind="Internal").ap()
    z_out = nc.dram_tensor("z_slice", [dstn, c_out], out_dt,
                           kind="ExternalOutput").ap()

    CH_A = 4

    def _env2(name, default):
        v = os.environ.get(name, default).split(",")
        return int(v[0] if chm or len(v) == 1 else v[1])

    ebufs = _env2("GAT_EBUFS", "2")
    mbufs = _env2("GAT_MBUFS", "2")
    gbufs = _env2("GAT_GBUFS", "4")

    with tile.TileContext(nc) as tc:
        with tc.tile_pool(name="const", bufs=1) as cpool, \
             tc.tile_pool(name="densex", bufs=2) as xpool, \
             tc.tile_pool(name="denseo", bufs=4) as opool, \
             tc.tile_pool(name="meta", bufs=mbufs) as mpool, \
             tc.tile_pool(name="edge", bufs=ebufs) as epool, \
             tc.tile_pool(name="sel", bufs=4) as spool, \
             tc.tile_pool(name="epi", bufs=2) as zpool, \
             tc.tile_pool(name="psA", bufs=4, space="PSUM") as ppa, \
             tc.tile_pool(name="psB", bufs=2, space="PSUM") as ppb:

            wext_t = []
            for k in range(kt):
                wtk = cpool.tile([P, wcols], BF16, tag=f"wext{k}")
                nc.sync.dma_start(out=wtk[:], in_=wext[k * P:(k + 1) * P, :])
                wext_t.append(wtk)
            iota_t = cpool.tile([P, P], BF16, tag="iota")
            nc.sync.dma_start(out=iota_t[:], in_=iota_d[:])
            iden_t = cpool.tile([P, P], BF16, tag="iden")
            nc.sync.dma_start(out=iden_t[:], in_=iden_d[:])
            ones_t = cpool.tile([P, 1], BF16, tag="ones")
            nc.vector.tensor_scalar(out=ones_t[:], in0=iota_t[:, 0:1],
                                    scalar1=0.0, scalar2=1.0,
                                    op0=mybir.AluOpType.mult,
                                    op1=mybir.AluOpType.add)
            bias_t = None
            if has_bias:
                bias_t = cpool.tile([P, c_out], F32, tag="bias")
                nc.sync.dma_start(out=bias_t[:], in_=bias_b[:])

            def _phase_a():
                # H rows [h | es]; ed ignored here
                for c0 in range(0, mt, CH_A):
                    c1 = min(c0 + CH_A, mt)
                    xc = []
                    for k in range(kt):
                        xck = xpool.tile([P, (c1 - c0) * P], BF16,
                                         tag=f"xc{k}")
                        nc.sync.dma_start(
                            out=xck[:],
                            in_=xT[k * P:(k + 1) * P, c0 * P:c1 * P])
                        xc.append(xck)
                    for m in range(c0, c1):
                        ps = ppa.tile([P, c_out + heads], F32, space="PSUM",
                                      tag="psA", bufs=2)
                        for k in range(kt):
                            nc.tensor.matmul(
                                out=ps[:],
                                lhsT=xc[k][:, (m - c0) * P:(m - c0 + 1) * P],
                                rhs=wext_t[k][:, :c_out + heads],
                                start=(k == 0), stop=(k == kt - 1))
                        # full-width row tile: the pad tail is zeroed once per
                        # slot below; full-width DRAM writes stay contiguous
                        # (strided row writes of this size crash the device)
                        ho = opool.tile([P, gcol], BF16, tag="ho")
                        if m % 2 == 0:
                            nc.vector.tensor_copy(out=ho[:, :c_out + heads],
                                                  in_=ps[:])
                        else:
                            nc.scalar.copy(out=ho[:, :c_out + heads],
                                           in_=ps[:])
                        if m < HALF // P:
                            nc.sync.dma_start(
                                out=hext_lo[m * P:(m + 1) * P, :], in_=ho[:])
                        else:
                            mm = m - HALF // P
                            nc.sync.dma_start(
                                out=hext_hi[mm * P:(mm + 1) * P, :],
                                in_=ho[:])

            def _phase_a2():
                # ED_local rows [ed] from the core's own x slice
                xs_t = []
                for k in range(kt):
                    xsk = xpool.tile([P, dstn_pad], BF16, tag=f"xs{k}")
                    nc.sync.dma_start(out=xsk[:],
                                      in_=xsT[k * P:(k + 1) * P, :])
                    xs_t.append(xsk)
                for t in range(wt):
                    ps = ppa.tile([P, heads], F32, space="PSUM", tag="psA2",
                                  bufs=1)
                    for k in range(kt):
                        nc.tensor.matmul(
                            out=ps[:],
                            lhsT=xs_t[k][:, t * P:(t + 1) * P],
                            rhs=wext_t[k][:, c_out + heads:wcols],
                            start=(k == 0), stop=(k == kt - 1))
                    ho = opool.tile([P, heads], BF16, tag="hoed")
                    nc.vector.tensor_copy(out=ho[:], in_=ps[:])
                    nc.sync.dma_start(out=edl[t * P:(t + 1) * P, :heads],
                                      in_=ho[:])

            def _phase_b():
                AL = mybir.AluOpType
                stage = int(os.environ.get("GAT_STAGE", "9"))
                if stage < 1:
                    return
                for (w0, w1, j0, nblk) in groups:
                    # meta chunk for this group
                    gix = mpool.tile([P, nblk * 8], I16, tag="m_gix")
                    nc.sync.dma_start(out=gix[:],
                                      in_=gidx_d[:, j0 * 8:(j0 + nblk) * 8])
                    loc = mpool.tile([P, nblk], BF16, tag="m_loc")
                    nc.sync.dma_start(out=loc[:], in_=loc_d[:, j0:j0 + nblk])
                    jw = 0  # block offset within group
                    for w in range(w0, w1):
                        nb_lo, nb_hi = wins[w]
                        nb = nb_lo + nb_hi
                        wr = min(P, dstn - w * P)
                        gt = epool.tile([P, nb, gcol], BF16, tag="g",
                                        bufs=gbufs)
                        if nb_lo:
                            nc.gpsimd.dma_gather(
                                out_ap=gt[:, :nb_lo, :],
                                in_ap=hext_lo[:],
                                idxs_ap=gix[:, jw * 8:(jw + nb_lo) * 8],
                                num_idxs=nb_lo * P, num_idxs_reg=nb_lo * P,
                                elem_size=gcol, single_packet=False)
                        if nb_hi:
                            nc.gpsimd.dma_gather(
                                out_ap=gt[:, nb_lo:, :],
                                in_ap=hext_hi[:],
                                idxs_ap=gix[:, (jw + nb_lo) * 8:
                                            (jw + nb) * 8],
                                num_idxs=nb_hi * P, num_idxs_reg=nb_hi * P,
                                elem_size=gcol, single_packet=False)
                        if stage < 3:
                            if stage >= 2:
                                # tiny consumer: forces gather completion
                                dmy = zpool.tile([P, nb * 2], BF16,
                                                 tag="dmy")
                                nc.vector.tensor_copy(
                                    out=dmy[:].rearrange(
                                        "p (n c) -> p n c", c=2),
                                    in_=gt[:, :, 0:2])
                            jw += nb
                            continue
                        # dst-side logits for this window's 128 dsts
                        edwin = epool.tile([P, P], BF16, tag="edw")
                        nc.sync.dma_start(out=edwin[:],
                                          in_=edl[w * P:(w + 1) * P, :])
                        # one-hot S01[e, d]: all the window's blocks in one op
                        s01w = spool.tile([P, nb, P], BF16, tag="s01w")
                        s01tw = spool.tile([P, nb, P], BF16, tag="s01tw")
                        psed = ppb.tile([P, nb * heads], F32, space="PSUM",
                                        tag="psED",
                                        bufs=int(os.environ.get("GAT_PSED",
                                                                "1")))
                        nc.vector.tensor_tensor(
                            out=s01w[:],
                            in0=loc[:, jw:jw + nb, None].to_broadcast(
                                (P, nb, P)),
                            in1=iota_t[:, None, :].to_broadcast((P, nb, P)),
                            op=AL.is_equal)
                        TC = 4   # transposed blocks per PSUM tile / ACT copy
                        for b0 in range(0, nb, TC):
                            b1 = min(b0 + TC, nb)
                            pst = ppa.tile([P, TC, P], BF16, space="PSUM",
                                           tag="psT", bufs=2)
                            for b in range(b0, b1):
                                nc.tensor.transpose(out=pst[:, b - b0, :],
                                                    in_=s01w[:, b, :],
                                                    identity=iden_t[:])
                            nc.scalar.copy(out=s01tw[:, b0:b1, :],
                                           in_=pst[:, :b1 - b0, :])
                        for b in range(nb):
                            nc.tensor.matmul(
                                out=psed[:, b * heads:(b + 1) * heads],
                                lhsT=s01tw[:, b, :],
                                rhs=edwin[:, :heads],
                                start=True, stop=True)
                        # ed per edge -> sbuf, then p = exp(lrelu(es+ed))
                        ed_sb = epool.tile([P, nb * heads], BF16, tag="edsb")
                        nc.scalar.copy(out=ed_sb[:], in_=psed[:])
                        e_t = epool.tile([P, nb * heads], BF16, tag="e")
                        nc.vector.tensor_tensor(
                            out=e_t[:].rearrange("p (n h) -> p n h", h=heads),
                            in0=gt[:, :, c_out:c_out + heads],
                            in1=ed_sb[:].rearrange("p (n h) -> p n h",
                                                   h=heads),
                            op=AL.add)
                        el = epool.tile([P, nb * heads], BF16, tag="el")
                        nc.vector.tensor_scalar_mul(el[:], e_t[:], NEG_SLOPE)
                        nc.vector.tensor_tensor(out=el[:], in0=e_t[:],
                                                in1=el[:], op=AL.max)
                        p_t = epool.tile([P, nb * heads], BF16, tag="p")
                        nc.scalar.activation(
                            out=p_t[:], in_=el[:],
                            func=mybir.ActivationFunctionType.Exp)
                        p3 = p_t[:].rearrange("p (n h) -> p n h", h=heads)
                        if chm:
                            # v = [h * p (channel-major) | p]
                            v_t = epool.tile([P, nb, dcol], BF16, tag="v")
                            nc.vector.tensor_tensor(
                                out=v_t[:, :, :c_out].rearrange(
                                    "p n (c h) -> p n c h", h=heads),
                                in0=gt[:, :, :c_out].rearrange(
                                    "p n (c h) -> p n c h", h=heads),
                                in1=p3[:, :, None, :].to_broadcast(
                                    (P, nb, c_out // heads, heads)),
                                op=AL.mult)
                            nc.vector.tensor_copy(
                                out=v_t[:, :, c_out:dcol], in_=p3)
                        else:
                            # ones into the es column -> psum col = sum p,
                            # then v = row * p (per-edge scalar broadcast)
                            nc.vector.tensor_copy(
                                out=gt[:, :, c_out:c_out + 1],
                                in_=ones_t[:, None, :].to_broadcast(
                                    (P, nb, 1)))
                            v_t = epool.tile([P, nb, dcol], BF16, tag="v")
                            nc.vector.tensor_tensor(
                                out=v_t[:],
                                in0=gt[:, :, :dcol],
                                in1=p3.to_broadcast((P, nb, dcol)),
                                op=AL.mult)
                        ps = ppb.tile([P, dcol], F32, space="PSUM", tag="psB")
                        for b in range(nb):
                            nc.tensor.matmul(out=ps[:], lhsT=s01w[:, b, :],
                                             rhs=v_t[:, b, :],
                                             start=(b == 0),
                                             stop=(b == nb - 1))
                        jw += nb
                        # ---- window epilogue ----
                        if stage < 6:
                            zt0 = zpool.tile([P, c_out], F32, tag="zt")
                            nc.vector.tensor_copy(out=zt0[:],
                                                  in_=ps[:, :c_out])
                            zf0 = zpool.tile([P, c_out], out_dt, tag="zf")
                            nc.vector.tensor_copy(out=zf0[:], in_=zt0[:])
                            nc.sync.dma_start(
                                out=z_out[w * P:w * P + wr, :],
                                in_=zf0[:wr, :])
                            continue
                        rec = zpool.tile([P, heads], F32, tag="rec")
                        nc.vector.reciprocal(out=rec[:],
                                             in_=ps[:, c_out:dcol])
                        zt = zpool.tile([P, c_out], F32, tag="zt")
                        if chm:
                            # un-permute channel-major -> head-major
                            nc.vector.tensor_tensor(
                                out=zt[:].rearrange(
                                    "p (h c) -> p h c", h=heads),
                                in0=bass.AP(
                                    ps.tensor, ps.offset,
                                    [ps.ap[0], [1, heads],
                                     [heads, c_out // heads]]),
                                in1=rec[:, :, None].to_broadcast(
                                    (P, heads, c_out // heads)),
                                op=AL.mult)
                        else:
                            nc.vector.tensor_tensor(
                                out=zt[:], in0=ps[:, :c_out],
                                in1=rec[:, 0:1].to_broadcast((P, c_out)),
                                op=AL.mult)
                        if has_bias:
                            nc.vector.tensor_add(out=zt[:], in0=zt[:],
                                                 in1=bias_t[:])
                        tneg = zpool.tile([P, c_out], F32, tag="tneg")
                        nc.vector.tensor_scalar_min(tneg[:], zt[:], 0.0)
                        texp = zpool.tile([P, c_out], F32, tag="texp")
                        nc.scalar.activation(
                            out=texp[:], in_=tneg[:],
                            func=mybir.ActivationFunctionType.Exp)
                        tpos = zpool.tile([P, c_out], F32, tag="tpos")
                        nc.vector.tensor_scalar(out=tpos[:], in0=zt[:],
                                                scalar1=0.0, scalar2=-1.0,
                                                op0=AL.max, op1=AL.add)
                        zf = zpool.tile([P, c_out], out_dt, tag="zf")
                        nc.vector.tensor_add(out=zf[:], in0=texp[:],
                                             in1=tpos[:])
                        nc.sync.dma_start(out=z_out[w * P:w * P + wr, :],
                                          in_=zf[:wr, :])

            skip = set(filter(None, os.environ.get("GAT_SKIP", "").split(",")))
            for _rep in range(work_mult):
                if "A2" not in skip:
                    _phase_a2()
                if "A" not in skip:
                    _phase_a()
                _phase_b()

    nc.compile()
    return nc


# --------------------------------------------------------------------------
# runner
# --------------------------------------------------------------------------

def _pjrt_run(nc, in_maps, repeat=1, time_it=False):
    """Run the compiled Bass program on all 8 cores via PJRT/shard_map.
    Returns (results, call_again())."""
    import time as _time

    import jax
    from jax.sharding import Mesh, NamedSharding, PartitionSpec
    from jax.experimental.shard_map import shard_map
    from concourse import bass2jax as b2j
    from concourse import mybir as _mb

    b2j.install_neuronx_cc_hook()

    part_name = (nc.partition_id_tensor.name
                 if nc.partition_id_tensor is not None else None)
    in_names, out_names, out_avals, zero_outs = [], [], [], []
    for alloc in nc.m.functions[0].allocations:
        if not isinstance(alloc, _mb.MemoryLocationSet):
            continue
        name = alloc.memorylocations[0].name
        if alloc.kind == "ExternalInput":
            if name != part_name:
                in_names.append(name)
        elif alloc.kind == "ExternalOutput":
            out_names.append(name)
            shape = tuple(alloc.tensor_shape)
            dtype = _mb.dt.np(alloc.dtype)
            out_avals.append(jax.core.ShapedArray(shape, dtype))
            zero_outs.append(np.zeros(shape, dtype))
    n_params = len(in_names)
    all_names = in_names + out_names
    if part_name is not None:
        all_names = all_names + [part_name]

    def _body(*args):
        operands = list(args)
        if part_name is not None:
            operands.append(b2j.partition_id_tensor())
        outs = b2j._bass_exec_p.bind(
            *operands,
            out_avals=tuple(out_avals),
            in_names=tuple(all_names),
            out_names=tuple(out_names),
            lowering_input_output_aliases=(),
            sim_require_finite=True,
            sim_require_nnan=True,
            nc=nc,
        )
        return tuple(outs)

    devices = jax.devices()[:NCORES]
    mesh = Mesh(np.asarray(devices), ("core",))
    nio = n_params + len(out_names)
    sharded = jax.jit(
        shard_map(_body, mesh=mesh,
                  in_specs=(PartitionSpec("core"),) * nio,
                  out_specs=(PartitionSpec("core"),) * len(out_names),
                  check_rep=False),
        keep_unused=True,
    )
    sh = NamedSharding(mesh, PartitionSpec("core"))
    concat_in = [
        jax.device_put(
            np.concatenate([np.asarray(in_maps[c][nm])
                            for c in range(NCORES)], axis=0), sh)
        for nm in in_names
    ]
    concat_zeros = [
        jax.device_put(
            np.zeros((NCORES * z.shape[0], *z.shape[1:]), z.dtype), sh)
        for z in zero_outs
    ]
    out_arrs = sharded(*concat_in, *concat_zeros)   # compile + first run
    jax.block_until_ready(out_arrs)

    def call_again():
        t0 = _time.perf_counter()
        o = sharded(*concat_in, *concat_zeros)
        jax.block_until_ready(o)
        return _time.perf_counter() - t0

    results = [
        {nm: np.asarray(out_arrs[i]).reshape(NCORES, *out_avals[i].shape)[c]
         for i, nm in enumerate(out_names)}
        for c in range(NCORES)
    ]
    return results, call_again


def _run(nc, in_maps):
    if os.environ.get("GAT_TIME", "0") == "1":
        nsamp = int(os.environ.get("GAT_SAMPLES", "7"))
        results, call1 = _pjrt_run(nc, in_maps)
        call1(); call1()          # warmup
        t1 = min(call1() for _ in range(nsamp))
        LAST_EXEC_NS.append(t1 * 1e9)
        print(f"[timing] call wall {t1*1e3:.3f} ms")
        return results
    res = run_bass_kernel_spmd(nc, in_maps, list(range(NCORES)), trace=False)
    return res.results


# --------------------------------------------------------------------------
# main entry
# --------------------------------------------------------------------------

def kernel(x, edge_index, W1, att_src1, att_dst1, b1, W2, att_src2, att_dst2,
           b2, **_ignored):
    x = np.asarray(x, dtype=np.float32)
    edge_index = np.asarray(edge_index)
    n = x.shape[0]
    fin = x.shape[1]
    c = np.asarray(W1).shape[1]
    heads1, _ = np.asarray(att_src1).shape
    heads2, _ = np.asarray(att_src2).shape
    assert n % NCORES == 0
    dstn = n // NCORES
    dstn_pad = math.ceil(dstn / P) * P
    n_pad = math.ceil(n / P) * P

    LAST_EXEC_NS.clear()

    loops = np.arange(n, dtype=np.int64)
    src = np.concatenate([np.asarray(edge_index[0]), loops])
    dst = np.concatenate([np.asarray(edge_index[1]), loops])
    per_core, wins, groups, tb = _preprocess_edges(src, dst, n, dstn)

    iota = np.broadcast_to(np.arange(P, dtype=np.float32),
                           (P, P)).astype(NPBF).copy()
    iden = np.eye(P, dtype=np.float32).astype(NPBF)
    w1ext = _fold_weights(np.asarray(W1), np.asarray(att_src1),
                          np.asarray(att_dst1)).astype(NPBF)
    w2ext = _fold_weights(np.asarray(W2), np.asarray(att_src2),
                          np.asarray(att_dst2)).astype(NPBF)
    b1 = np.asarray(b1, dtype=np.float32)
    b2 = np.asarray(b2, dtype=np.float32)
    has_b1 = bool(np.any(b1 != 0.0))
    has_b2 = bool(np.any(b2 != 0.0))
    work_mult = int(os.environ.get("GAT_WORKMULT", "1"))

    def make_maps(xTb, wextb, hasb, bvec, cdim):
        maps = []
        for k in range(NCORES):
            xs = np.ascontiguousarray(
                xTb[:, k * dstn:(k + 1) * dstn])
            xsp = np.zeros((xTb.shape[0], dstn_pad), NPBF)
            xsp[:, :dstn] = xs
            m = dict(xT=xTb, xsliceT=xsp, wext=wextb, iota=iota, iden=iden,
                     gidx=per_core[k]["gidx"],
                     dstloc=per_core[k]["dstloc"])
            if hasb:
                m["bias_b"] = np.broadcast_to(bvec, (P, cdim)).copy()
            maps.append(m)
        return maps

    # ---- layer 1 ----
    xT = np.zeros((fin, n_pad), NPBF)
    xT[:, :n] = x.T.astype(NPBF)
    nc1 = _build_layer(n_pad=n_pad, in_k=fin, c_out=c, heads=heads1,
                       tb=tb, wins=wins, groups=groups, dstn=dstn,
                       has_bias=has_b1, out_dt=BF16, work_mult=work_mult)
    res1 = _run(nc1, make_maps(xT, w1ext, has_b1, b1, c))
    z = np.concatenate([res1[k]["z_slice"] for k in range(NCORES)], axis=0)

    # ---- layer 2 ----
    c2 = np.asarray(W2).shape[1]
    zT = np.zeros((c, n_pad), NPBF)
    zT[:, :n] = np.asarray(z, NPBF).T
    nc2 = _build_layer(n_pad=n_pad, in_k=c, c_out=c2, heads=heads2,
                       tb=tb, wins=wins, groups=groups, dstn=dstn,
                       has_bias=has_b2, out_dt=F32, work_mult=work_mult)
    res2 = _run(nc2, make_maps(zT, w2ext, has_b2, b2, c2))
    out = np.concatenate([res2[k]["z_slice"] for k in range(NCORES)], axis=0)
    return out.astype(np.float32)

